# revision 1
# baseline (speedup 1.0000x reference)
"""GAT layer (N=50000, E=1.6M, D=128, H=4) on 8 trn2 NeuronCores.

Strategy: node partition. Edges are sorted by src; each core owns 49
blocks of 128 nodes plus every edge leaving them (~E/8, balanced). Per
128-edge tile, one indirect DMA gathers augmented rows
[feat(128) | 1 | s_nbr(4) | pad] for the edges' dst nodes; fused
tensor_scalar ops build per-head ex-scaled one-hot matrices over the
src block; TensorE matmuls accumulate [G_h | denom_h] in PSUM.
Attention scores, softmax normalization, head mix, relu, gate and
residual are dense per-block ops. No collectives are needed.
"""

import numpy as np

import concourse.bass as bass
import concourse.bacc as bacc
import concourse.mybir as mybir
import concourse.tile as tile
from concourse.bass_utils import run_bass_kernel_spmd
from concourse.masks import make_identity

N = 50000
E = 1_600_000
D = 128
H = 4
LEAKY = 0.2
N_CORES = 8
P = 128
BC = 49  # blocks per core
NB = N_CORES * BC  # 392
NPAD = NB * P  # 50176
FAW = 136  # FA row: feat(128) | one(1) | s_nbr(4) | pad(3)
GSUP = 8
F32 = mybir.dt.float32
I32 = mybir.dt.int32

_cache = {}


def _host_prep(edge_index):
    idx = np.asarray(edge_index).reshape(-1, 2)
    src = idx[:, 0].astype(np.int64)
    dst = idx[:, 1].astype(np.int32)
    order = np.argsort(src, kind="stable")
    src_s = src[order].astype(np.int32)
    dst_s = dst[order]
    blk = src_s >> 7
    cnt = np.bincount(blk, minlength=NB)
    starts = np.concatenate([[0], np.cumsum(cnt)])
    cnt2 = cnt.reshape(N_CORES, BC)
    T_b = np.maximum(1, -(-cnt2.max(axis=0) // P)).astype(int)  # [BC]
    offs = np.concatenate([[0], np.cumsum(T_b)]).astype(int)
    CT = int(offs[-1])

    dst_i = np.zeros((N_CORES, P, CT), np.int32)
    src_l = np.full((N_CORES, P, CT), 999.0, np.float32)
    for c in range(N_CORES):
        for lb in range(BC):
            b = c * BC + lb
            n = int(cnt[b])
            t = int(T_b[lb])
            dpad = np.zeros(t * P, np.int32)
            spad = np.full(t * P, 999.0, np.float32)
            dpad[:n] = dst_s[starts[b] : starts[b] + n]
            spad[:n] = (src_s[starts[b] : starts[b] + n] - b * P).astype(np.float32)
            o = offs[lb]
            dst_i[c, :, o : o + t] = dpad.reshape(t, P).T
            src_l[c, :, o : o + t] = spad.reshape(t, P).T
    return T_b, offs, CT, dst_i, src_l


def _build(T_b, offs, CT, repeat=1):
    nc = bacc.Bacc("TRN2", target_bir_lowering=False, debug=False, num_devices=N_CORES)

    fa0 = nc.dram_tensor("fa0", [NPAD, 129], F32, kind="ExternalInput").ap()
    featc = nc.dram_tensor("featc", [BC * P, D], F32, kind="ExternalInput").ap()
    dsti = nc.dram_tensor("dsti", [P, CT], I32, kind="ExternalInput").ap()
    srcl = nc.dram_tensor("srcl", [P, CT], F32, kind="ExternalInput").ap()
    skt = nc.dram_tensor("skt", [H, D], F32, kind="ExternalInput").ap()
    akt = nc.dram_tensor("akt", [H, 2 * D], F32, kind="ExternalInput").ap()
    gwt = nc.dram_tensor("gwt", [D, D], F32, kind="ExternalInput").ap()
    gbt = nc.dram_tensor("gbt", [1, D], F32, kind="ExternalInput").ap()
    outp = nc.dram_tensor("outp", [BC * P, D], F32, kind="ExternalOutput").ap()

    ACT = mybir.ActivationFunctionType
    ALU = mybir.AluOpType

    with tile.TileContext(nc) as tc:
        with (
            tc.tile_pool(name="const", bufs=1) as cp,
            tc.tile_pool(name="work", bufs=2) as wp,
            tc.tile_pool(name="dram", bufs=1, space="DRAM") as dp,
            tc.tile_pool(name="psum", bufs=1, space="PSUM") as pp,
        ):
            FA = dp.tile([NPAD, FAW], F32)

            # ---- constants ----
            ident = cp.tile([P, P], F32)
            make_identity(nc, ident[:])
            iota_b = cp.tile([P, P], F32)
            nc.gpsimd.iota(
                iota_b[:],
                pattern=[[1, P]],
                base=0,
                channel_multiplier=0,
                allow_small_or_imprecise_dtypes=True,
            )
            ones_r = cp.tile([1, P], F32)
            nc.vector.memset(ones_r[:], 1.0)
            sk = cp.tile([H, D], F32)
            ak = cp.tile([H, 2 * D], F32)
            gw = cp.tile([D, D], F32)
            gb = cp.tile([1, D], F32)
            nc.sync.dma_start(sk[:], skt[:])
            nc.sync.dma_start(ak[:], akt[:])
            nc.sync.dma_start(gw[:], gwt[:])
            nc.sync.dma_start(gb[:], gbt[:])

            # W_cat [D, 2H]: col h = sk_h*ak_h[:D], col H+h = sk_h*ak_h[D:]
            wself = cp.tile([H, D], F32)
            wnbr = cp.tile([H, D], F32)
            nc.vector.tensor_mul(wself[:], sk[:], ak[:, 0:D])
            nc.vector.tensor_mul(wnbr[:], sk[:], ak[:, D : 2 * D])
            wcat = cp.tile([D, 2 * H], F32)
            for j, w in enumerate((wself, wnbr)):
                wc_ps = pp.tile([P, P], F32, tag="trans", bufs=2, space="PSUM")
                nc.tensor.transpose(wc_ps[:, 0:H], w[:], ident[0:H, 0:H])
                nc.scalar.activation(
                    wcat[:, j * H : (j + 1) * H], wc_ps[0:D, 0:H], ACT.Copy
                )

            skb = []
            for h in range(H):
                skrow = cp.tile([1, D], F32, tag=f"skrow{h}", name=f"skrow{h}")
                nc.sync.dma_start(skrow[:], skt[h : h + 1, :])
                sb_ps = pp.tile([P, P], F32, tag="trans", bufs=2, space="PSUM")
                nc.tensor.matmul(
                    sb_ps[:, 0:D], ones_r[:], skrow[:], start=True, stop=True
                )
                skbh = cp.tile([P, D], F32, tag=f"skb{h}", name=f"skb{h}")
                nc.scalar.activation(skbh[:], sb_ps[:, 0:D], ACT.Copy)
                skb.append(skbh)

            def _phases():
                # ---- Phase A-own: s_self + resident featT for own blocks ----
                s_self_own = cp.tile([P, BC * H], F32)
                ftr_own = cp.tile([P, BC * D], F32)
                for lb in range(BC):
                    fco = wp.tile([P, D], F32, tag="fblk")
                    nc.sync.dma_start(fco[:], featc[lb * P : (lb + 1) * P, :])
                    t_ps = pp.tile([P, P], F32, tag="trans", bufs=2, space="PSUM")
                    nc.tensor.transpose(t_ps[:], fco[:], ident[:])
                    nc.scalar.activation(
                        ftr_own[:, lb * D : (lb + 1) * D], t_ps[:], ACT.Copy
                    )
                    ss_ps = pp.tile([P, GSUP * H], F32, tag="spsum", bufs=2, space="PSUM")
                    nc.tensor.matmul(
                        ss_ps[:, 0:H],
                        ftr_own[:, lb * D : (lb + 1) * D],
                        wcat[:, 0:H],
                        start=True,
                        stop=True,
                    )
                    nc.vector.tensor_copy(
                        s_self_own[:, lb * H : (lb + 1) * H], ss_ps[:, 0:H]
                    )

                # ---- Phase A-all: build FA rows (feat|1|s_nbr) for all blocks ----
                for b in range(NB):
                    fb = wp.tile([P, 129], F32, tag="fb129")
                    nc.sync.dma_start(fb[:], fa0[b * P : (b + 1) * P, :])
                    t_ps = pp.tile([P, P], F32, tag="trans", bufs=2, space="PSUM")
                    nc.tensor.transpose(t_ps[:], fb[:, 0:D], ident[:])
                    ftb = wp.tile([P, D], F32, tag="ftb")
                    nc.scalar.activation(ftb[:], t_ps[:], ACT.Copy)
                    sn_ps = pp.tile([P, GSUP * H], F32, tag="spsum", bufs=2, space="PSUM")
                    nc.tensor.matmul(
                        sn_ps[:, 0:H], ftb[:], wcat[:, H : 2 * H], start=True, stop=True
                    )
                    fab = wp.tile([P, FAW], F32, tag="fab")
                    nc.vector.tensor_copy(fab[:, 0:129], fb[:])
                    nc.scalar.activation(fab[:, 129:133], sn_ps[:, 0:H], ACT.Copy)
                    nc.vector.memset(fab[:, 133:FAW], 0.0)
                    nc.sync.dma_start(FA[b * P : (b + 1) * P, :], fab[:])

                # ---- Phases B/C/D per own block ----
                for lb in range(BC):
                    T = int(T_b[lb])
                    off = int(offs[lb])
                    gps = [
                        pp.tile(
                            [P, 129], F32, tag=f"G{h}", bufs=1, space="PSUM", name=f"g{h}"
                        )
                        for h in range(H)
                    ]
                    sblk = s_self_own[:, lb * H : (lb + 1) * H]
                    dcol = wp.tile([P, T], I32, tag="dcol")
                    nc.sync.dma_start(dcol[:], dsti[:, off : off + T])
                    scol = wp.tile([P, T], F32, tag="scol")
                    nc.sync.dma_start(scol[:], srcl[:, off : off + T])

                    ti = 0
                    while ti < T:
                        g = min(GSUP, T - ti)
                        fg = wp.tile([P, GSUP, FAW], F32, tag="fg", bufs=4)
                        sps = pp.tile([P, GSUP * H], F32, tag="spsum", bufs=2, space="PSUM")
                        ohs = []
                        for tt in range(g):
                            t = ti + tt
                            nc.gpsimd.indirect_dma_start(
                                out=fg[:, tt, :],
                                out_offset=None,
                                in_=FA[:],
                                in_offset=bass.IndirectOffsetOnAxis(
                                    ap=dcol[:, t : t + 1], axis=0
                                ),
                            )
                            oh = wp.tile([P, P], F32, tag="oh", bufs=GSUP + 2, name="oh")
                            nc.vector.tensor_scalar(
                                oh[:], iota_b[:], scol[:, t : t + 1], None, ALU.is_equal
                            )
                            ohs.append(oh)
                            t_ps = pp.tile([P, P], F32, tag="trans", bufs=2, space="PSUM")
                            nc.tensor.transpose(t_ps[:], oh[:], ident[:])
                            ohT = wp.tile([P, P], F32, tag="ohT", bufs=3, name="ohT")
                            nc.scalar.activation(ohT[:], t_ps[:], ACT.Copy)
                            nc.tensor.matmul(
                                sps[:, tt * H : (tt + 1) * H],
                                ohT[:],
                                sblk,
                                start=True,
                                stop=True,
                            )
                        eraw = wp.tile([P, GSUP * H], F32, tag="eraw")
                        nc.vector.tensor_add(
                            eraw[:, 0 : g * H], sps[:, 0 : g * H], fg[:, 0:g, 129:133]
                        )
                        elr = wp.tile([P, GSUP * H], F32, tag="elr")
                        nc.scalar.activation(
                            elr[:, 0 : g * H], eraw[:, 0 : g * H], ACT.Prelu, alpha=LEAKY
                        )
                        ex = wp.tile([P, GSUP * H], F32, tag="ex")
                        nc.scalar.activation(ex[:, 0 : g * H], elr[:, 0 : g * H], ACT.Exp)
                        for tt in range(g):
                            t = ti + tt
                            for h in range(H):
                                c = tt * H + h
                                ohx = wp.tile(
                                    [P, P], F32, tag="ohx", bufs=GSUP, name="ohx"
                                )
                                nc.vector.tensor_scalar(
                                    ohx[:], ohs[tt][:], ex[:, c : c + 1], None, ALU.mult
                                )
                                nc.tensor.matmul(
                                    gps[h][:],
                                    ohx[:],
                                    fg[:, tt, 0:129],
                                    start=(t == 0),
                                    stop=(t == T - 1),
                                    skip_group_check=True,
                                )
                        ti += g

                    # epilogue: rec = 1/(H*max(den,eps)); mix heads; relu
                    den = wp.tile([P, H], F32, tag="den")
                    for h in range(H):
                        nc.vector.tensor_copy(den[:, h : h + 1], gps[h][:, 128:129])
                    den2 = wp.tile([P, H], F32, tag="den2")
                    nc.vector.tensor_scalar(
                        den2[:], den[:], 1.0e-30, float(H), ALU.max, ALU.mult
                    )
                    rec = wp.tile([P, H], F32, tag="rec")
                    nc.vector.reciprocal(rec[:], den2[:])
                    acc = wp.tile([P, D], F32, tag="acc")
                    for h in range(H):
                        th = wp.tile([P, D], F32, tag="th")
                        nc.scalar.activation(
                            th[:], gps[h][:, 0:D], ACT.Copy, scale=rec[:, h : h + 1]
                        )
                        if h == 0:
                            nc.vector.tensor_mul(acc[:], th[:], skb[0][:])
                        else:
                            th2 = wp.tile([P, D], F32, tag="th2")
                            nc.vector.tensor_mul(th2[:], th[:], skb[h][:])
                            nc.vector.tensor_add(acc[:], acc[:], th2[:])
                    oagg = wp.tile([P, D], F32, tag="oagg")
                    nc.scalar.activation(oagg[:], acc[:], ACT.Relu)

                    # gate + residual
                    fco = wp.tile([P, D], F32, tag="fblk")
                    nc.sync.dma_start(fco[:], featc[lb * P : (lb + 1) * P, :])
                    g_ps = pp.tile([P, P], F32, tag="trans", bufs=2, space="PSUM")
                    nc.tensor.matmul(
                        g_ps[:, 0:D],
                        ftr_own[:, lb * D : (lb + 1) * D],
                        gw[:],
                        start=True,
                        stop=False,
                        skip_group_check=True,
                    )
                    nc.tensor.matmul(
                        g_ps[:, 0:D],
                        ones_r[:],
                        gb[:],
                        start=False,
                        stop=True,
                        skip_group_check=True,
                    )
                    gate = wp.tile([P, D], F32, tag="gate")
                    nc.scalar.activation(gate[:], g_ps[:, 0:D], ACT.Sigmoid)
                    dif = wp.tile([P, D], F32, tag="dif")
                    nc.vector.tensor_sub(dif[:], oagg[:], fco[:])
                    gd = wp.tile([P, D], F32, tag="gd")
                    nc.vector.tensor_mul(gd[:], gate[:], dif[:])
                    res = wp.tile([P, D], F32, tag="res")
                    nc.vector.tensor_add(res[:], fco[:], gd[:])
                    nc.sync.dma_start(outp[lb * P : (lb + 1) * P, :], res[:])

            if repeat == 1:
                _phases()
            else:
                with tc.For_i(0, repeat, 1):
                    _phases()

    nc.compile()
    return nc


def kernel(edge_index, features, self_kernels, attn_kernels, gate_weight, gate_bias):
    T_b, offs, CT, dst_i, src_l = _host_prep(edge_index)

    fa0 = np.zeros((NPAD, 129), np.float32)
    fa0[:N, 0:D] = np.asarray(features, np.float32)
    fa0[:, 128] = 1.0
    featf = fa0[:, 0:D]

    key = ("prog", CT, tuple(int(x) for x in T_b))
    if key not in _cache:
        _cache[key] = _build(T_b, offs, CT)
    nc = _cache[key]

    in_maps = []
    for c in range(N_CORES):
        in_maps.append(
            {
                "fa0": fa0,
                "featc": np.ascontiguousarray(
                    featf[c * BC * P : (c + 1) * BC * P, :]
                ),
                "dsti": dst_i[c],
                "srcl": src_l[c],
                "skt": np.asarray(self_kernels, np.float32),
                "akt": np.asarray(attn_kernels, np.float32),
                "gwt": np.asarray(gate_weight, np.float32),
                "gbt": np.asarray(gate_bias, np.float32).reshape(1, D),
            }
        )
    res = run_bass_kernel_spmd(nc, in_maps, core_ids=list(range(N_CORES)))
    out = np.concatenate([res.results[c]["outp"] for c in range(N_CORES)], axis=0)
    return out[:N].astype(np.float32)

